# revision 6
# baseline (speedup 1.0000x reference)
"""Trainium2 Bass kernel for the DifferentiableQuantumCircuit problem.

Math: output = |U x / ||x|| |^2 with U = kron of 12 single-qubit U3 gates
applied twice (2 layers). Gates on different qubits commute, so the two
layers fuse into ONE kron-product unitary with per-qubit gates
G_q = U3_layer2(q) @ U3_layer1(q).

State index split: i = q5 * 128 + l7, with q5 = qubits 0-4 (5 MSBs) and
l7 = qubits 5-11 (7 LSBs, contiguous in memory -> 512B DMA bursts).
U_total = M5a (x) M7b with M5a = kron(G_0..G_4) [32x32] acting on q5 and
M7b = kron(G_5..G_11) [128x128] acting on l7.

Per-core pipeline (512 samples/core, 4 chunks of 128 samples b=(bh,b2),
bh in [0,32), b2 in [0,4); chunks split into 2 halves of 16 bh each):
  1. DMA-load half: Xh[(b2,q5), (bh,l7)] = x[bh*4+b2, q5*128+l7]
  2. stage 1 (PE "trick" matmuls): stationary = Xh column-chunk (fixed
     bh), moving = [Re(G5bd^T) | Im(G5bd^T)] with G5bd = I4 (x) M5a
     acting on the (b2,q5) partition index -> psum[l7, (re/im,(b2',q5'))]
     (applies the 5-qubit gate group AND transposes l7 onto partitions)
  3. evacuate psum -> S1 group tiles with 1/||x_b|| fused (broadcast-AP
     multiply on VectorE)
  4. stage 2: stationary = S1r/S1i column-chunks [l7, (b2',q5')], moving
     = [Re(M7b^T)|Im(M7b^T)] / [-Im|Re], accumulating
     -> psum[(b2',q5'), (re/im, l7')]
  5. squares on ScalarE, re^2+im^2 add on GpSimd (per half)
  6. DMA-store Ph[(b2,q5'), (bh, l7')] -> out[b, i]  (512B bursts)

Norm chain (per chunk): x^2 (ScalarE) -> 128-segment reduce (VectorE) ->
block-diag-ones matmul (PE, sums over q5 per b2 group) -> tiny DMAs to a
single-partition row -> sqrt (ScalarE) -> reciprocal (VectorE) ->
ones-column matmul broadcast to all partitions (PE) -> TRBC tile.
"""

from contextlib import ExitStack

import numpy as np

import concourse.bass as bass
import concourse.tile as tile
from concourse import bacc, mybir
from concourse.bass_utils import run_bass_kernel_spmd

F32 = mybir.dt.float32
F32R = mybir.dt.float32r

NUM_QUBITS = 12
D = 4096
B = 4096
N_CORES = 8
B_CORE = B // N_CORES  # 512
CHUNK = 128
N_CHUNKS = B_CORE // CHUNK  # 4
GROUP = 4  # c-tiles per psum group tile (2 banks)
HALF = D // 2  # free columns per half-chunk (16 bh x 128 l7)


def _u3(theta, phi, lam):
    """Single-qubit U3 gate, complex128 [2,2] (same formula as reference)."""
    c = np.cos(theta / 2.0)
    s = np.sin(theta / 2.0)
    return np.array(
        [
            [c, -np.exp(1j * lam) * s],
            [np.exp(1j * phi) * s, np.exp(1j * (phi + lam)) * c],
        ],
        dtype=np.complex128,
    )


def _gate_consts(thetas, phis, lams):
    """Build the constant moving-operand matrices for both PE stages."""
    thetas = np.asarray(thetas, dtype=np.float64)
    phis = np.asarray(phis, dtype=np.float64)
    lams = np.asarray(lams, dtype=np.float64)
    gates = []
    for q in range(NUM_QUBITS):
        g1 = _u3(thetas[0, q], phis[0, q], lams[0, q])
        g2 = _u3(thetas[1, q], phis[1, q], lams[1, q])
        gates.append(g2 @ g1)  # layer 1 applied first, then layer 2

    m5a = gates[0]
    for q in range(1, 5):
        m5a = np.kron(m5a, gates[q])  # [32,32], acts on q5 (bits 0-4)
    m7b = gates[5]
    for q in range(6, 12):
        m7b = np.kron(m7b, gates[q])  # [128,128], acts on l7 (bits 5-11)

    g5 = np.kron(np.eye(4), m5a)  # [128,128] block-diag over (b2, q5)

    mv1 = np.concatenate([g5.T.real, g5.T.imag], axis=1)  # [128,256]
    mv2a = np.concatenate([m7b.T.real, m7b.T.imag], axis=1)
    mv2b = np.concatenate([-m7b.T.imag, m7b.T.real], axis=1)
    return (
        np.ascontiguousarray(mv1, dtype=np.float32),
        np.ascontiguousarray(mv2a, dtype=np.float32),
        np.ascontiguousarray(mv2b, dtype=np.float32),
    )


def _build_nc():
    nc = bacc.Bacc(
        "TRN2", target_bir_lowering=False, debug=False, num_devices=N_CORES
    )
    x_ap = nc.dram_tensor("x", [B_CORE, D], F32R, kind="ExternalInput").ap()
    mv1_ap = nc.dram_tensor("mv1", [128, 256], F32R, kind="ExternalInput").ap()
    mv2a_ap = nc.dram_tensor("mv2a", [128, 256], F32R, kind="ExternalInput").ap()
    mv2b_ap = nc.dram_tensor("mv2b", [128, 256], F32R, kind="ExternalInput").ap()
    out_ap = nc.dram_tensor("probs", [B_CORE, D], F32, kind="ExternalOutput").ap()

    with tile.TileContext(nc) as tc, ExitStack() as ctx:
        consts = ctx.enter_context(tc.tile_pool(name="consts", bufs=1))
        mv1_t = consts.tile([128, 256], F32R, tag="mv1")
        nc.sync.dma_start(mv1_t[:], mv1_ap[:])
        mv2a_t = consts.tile([128, 256], F32R, tag="mv2a")
        nc.sync.dma_start(mv2a_t[:], mv2a_ap[:])
        mv2b_t = consts.tile([128, 256], F32R, tag="mv2b")
        nc.sync.dma_start(mv2b_t[:], mv2b_ap[:])
        # I4 (x) ones32: sums over q5 within each b2 block
        bdones_t = consts.tile([128, 128], F32, tag="bdones")
        nc.vector.memset(bdones_t[:], 0.0)
        for b2 in range(4):
            s = slice(b2 * 32, (b2 + 1) * 32)
            nc.vector.memset(bdones_t[s, s], 1.0)
        # single-partition ones column for the partition-broadcast matmul
        onescol_t = consts.tile([1, 128], F32, tag="onescol")
        nc.vector.memset(onescol_t[:], 1.0)

        xpool = ctx.enter_context(tc.tile_pool(name="xp", bufs=4))
        bigp = ctx.enter_context(tc.tile_pool(name="bigp", bufs=8))
        smallp = ctx.enter_context(tc.tile_pool(name="smallp", bufs=2))
        s1pool = ctx.enter_context(tc.tile_pool(name="s1p", bufs=8))
        ppool = ctx.enter_context(tc.tile_pool(name="pp", bufs=4))
        ps1 = ctx.enter_context(tc.tile_pool(name="ps1", bufs=2, space="PSUM"))
        ps2 = ctx.enter_context(tc.tile_pool(name="ps2", bufs=2, space="PSUM"))

        for k in range(N_CHUNKS):
            # ---- load chunk halves + per-sample sum-of-squares
            xflat = x_ap[k * CHUNK : (k + 1) * CHUNK, :].flatten()
            seg = smallp.tile([128, 32], F32, tag="seg")
            Xh = []
            for h in range(2):
                X = xpool.tile([128, HALF], F32R, tag="X")
                Xh.append(X)
                nc.sync.dma_start(
                    X[:].rearrange("p (bh l) -> p bh l", l=128),
                    xflat[h * CHUNK * HALF : (h + 1) * CHUNK * HALF].rearrange(
                        "(bh p l) -> p bh l", p=128, l=128
                    ),
                )
                x2 = bigp.tile([128, HALF], F32, tag="big")
                nc.scalar.square(x2[:], X[:].bitcast(F32))
                nc.vector.tensor_reduce(
                    seg[:, h * 16 : (h + 1) * 16],
                    x2[:].rearrange("p (bh l) -> p bh l", l=128),
                    axis=mybir.AxisListType.X,
                    op=mybir.AluOpType.add,
                )

            # ---- norm chain -> TRBC[p, b2*32+bh] = 1/||x_(bh,b2)|| (all p)
            psv = ps1.tile([128, 32], F32, tag="g1")
            nc.tensor.matmul(
                psv[:], lhsT=bdones_t[:], rhs=seg[:], start=True, stop=True
            )
            psvs = smallp.tile([128, 32], F32, tag="psvs")
            nc.vector.tensor_copy(psvs[:], psv[:])
            # gather one row per b2 group onto partition 0: t1s[0, b2*32+bh]
            t1s = smallp.tile([1, 128], F32, tag="t1s")
            for b2 in range(4):
                nc.sync.dma_start(
                    t1s[0:1, b2 * 32 : (b2 + 1) * 32],
                    psvs[b2 * 32 : b2 * 32 + 1, :],
                )
            t1sq = smallp.tile([1, 128], F32, tag="t1sq")
            nc.scalar.sqrt(t1sq[:], t1s[:])
            t1inv = smallp.tile([1, 128], F32, tag="t1inv")
            nc.vector.reciprocal(t1inv[:], t1sq[:])
            # broadcast to all partitions via K=1 matmul
            psb = ps2.tile([128, 128], F32, tag="g2")
            nc.tensor.matmul(
                psb[:], lhsT=onescol_t[:], rhs=t1inv[:], start=True, stop=True
            )
            trbc = smallp.tile([128, 128], F32, tag="trbc")
            nc.vector.tensor_copy(trbc[:], psb[:])

            # ---- per half: stage 1 + stage 2 + squares, then add + store
            for h in range(2):
                X = Xh[h]
                T1 = bigp.tile([128, HALF], F32, tag="big")
                T2 = bigp.tile([128, HALF], F32, tag="big")
                for gl in range(4):  # groups within this half
                    g = h * 4 + gl
                    # stage 1 group
                    pg = ps1.tile([128, GROUP * 256], F32, tag="g1")
                    for j in range(GROUP):
                        cl = gl * GROUP + j  # c-tile local to half
                        nc.tensor.matmul(
                            pg[:, j * 256 : (j + 1) * 256],
                            lhsT=X[:, cl * 128 : (cl + 1) * 128],
                            rhs=mv1_t[:],
                            start=True,
                            stop=True,
                        )
                    # evacuate with 1/||x|| scaling (varies per (j, b2'))
                    S1r = s1pool.tile([128, GROUP * 128], F32R, tag="S1r")
                    S1i = s1pool.tile([128, GROUP * 128], F32R, tag="S1i")
                    pg4 = pg[:].rearrange(
                        "p (j r b2 q) -> p j r b2 q", j=GROUP, r=2, b2=4
                    )
                    vb = (
                        trbc[:]
                        .rearrange("p (b2 bh) -> p b2 bh", b2=4)[
                            :, :, g * GROUP : (g + 1) * GROUP
                        ]
                        .transpose([0, 2, 1])
                        .unsqueeze(3)
                        .broadcast_to([128, GROUP, 4, 32])
                    )
                    nc.vector.tensor_tensor(
                        S1r[:].rearrange("p (j b2 q) -> p j b2 q", j=GROUP, b2=4),
                        pg4[:, :, 0],
                        vb,
                        op=mybir.AluOpType.mult,
                    )
                    nc.vector.tensor_tensor(
                        S1i[:].rearrange("p (j b2 q) -> p j b2 q", j=GROUP, b2=4),
                        pg4[:, :, 1],
                        vb,
                        op=mybir.AluOpType.mult,
                    )
                    # stage 2 group
                    pg2 = ps2.tile([128, GROUP * 256], F32, tag="g2")
                    for j in range(GROUP):
                        cc = slice(j * 128, (j + 1) * 128)
                        nc.tensor.matmul(
                            pg2[:, j * 256 : (j + 1) * 256],
                            lhsT=S1r[:, cc],
                            rhs=mv2a_t[:],
                            start=True,
                            stop=False,
                        )
                        nc.tensor.matmul(
                            pg2[:, j * 256 : (j + 1) * 256],
                            lhsT=S1i[:, cc],
                            rhs=mv2b_t[:],
                            start=False,
                            stop=True,
                        )
                    pg3 = pg2[:].rearrange("p (j n) -> p j n", n=256)
                    gcols = slice(gl * GROUP * 128, (gl + 1) * GROUP * 128)
                    nc.scalar.square(
                        T1[:, gcols].rearrange("p (j n) -> p j n", n=128),
                        pg3[:, :, 0:128],
                    )
                    nc.scalar.square(
                        T2[:, gcols].rearrange("p (j n) -> p j n", n=128),
                        pg3[:, :, 128:256],
                    )

                # probs = re^2 + im^2 ; store back to [b, i] (512B bursts)
                P = ppool.tile([128, HALF], F32, tag="P")
                nc.gpsimd.tensor_tensor(
                    P[:], T1[:], T2[:], op=mybir.AluOpType.add
                )
                oflat = out_ap[k * CHUNK : (k + 1) * CHUNK, :].flatten()
                nc.sync.dma_start(
                    oflat[h * CHUNK * HALF : (h + 1) * CHUNK * HALF].rearrange(
                        "(bh p l) -> p bh l", p=128, l=128
                    ),
                    P[:].rearrange("p (bh l) -> p bh l", l=128),
                )

    nc.compile()
    return nc


_NC_CACHE = {}


def _get_nc():
    if "nc" not in _NC_CACHE:
        _NC_CACHE["nc"] = _build_nc()
    return _NC_CACHE["nc"]


def kernel(inputs, thetas, phis, lams, _trace=False, _trace_kwargs=None):
    inputs = np.ascontiguousarray(np.asarray(inputs), dtype=np.float32)
    mv1, mv2a, mv2b = _gate_consts(thetas, phis, lams)

    nc = _get_nc()
    in_maps = [
        {
            "x": inputs[k * B_CORE : (k + 1) * B_CORE],
            "mv1": mv1,
            "mv2a": mv2a,
            "mv2b": mv2b,
        }
        for k in range(N_CORES)
    ]
    res = run_bass_kernel_spmd(
        nc, in_maps, list(range(N_CORES)), trace=_trace, **(_trace_kwargs or {})
    )
    out = np.concatenate([res.results[k]["probs"] for k in range(N_CORES)], axis=0)
    if _trace:
        kernel.last_result = res
    return out
